# revision 1
# baseline (speedup 1.0000x reference)
"""GCN (2x GraphConv + BatchNorm) on 8 Trainium2 NeuronCores.

Sharding: 1D node partition (12500 dst-nodes per core). Edges are grouped by
dst shard on host (index preprocessing); each core gathers source features
from a replicated transformed-feature table, scatter-adds into its local node
block, and BN statistics are combined with psum collectives. Layer-2 input is
assembled with an all_gather.
"""
import numpy as np
from functools import partial

N = 100000
E = 1600000
F = 128
H = 64
EPS = 1e-5
NC = 8
NS = N // NC  # 12500 nodes per core


def _prep(src, dst):
    deg_out = np.bincount(src, minlength=N).astype(np.float32)
    deg_in = np.bincount(dst, minlength=N).astype(np.float32)
    norm_src = 1.0 / np.sqrt(np.maximum(deg_out, 1.0))
    norm_dst = 1.0 / np.sqrt(np.maximum(deg_in, 1.0))
    shard = dst // NS
    order = np.argsort(shard, kind="stable")
    s_sorted = src[order]
    d_sorted = dst[order]
    counts = np.bincount(shard, minlength=NC)
    L = int(counts.max())
    L = ((L + 127) // 128) * 128
    src_p = np.zeros((NC, L), np.int32)
    dstl_p = np.full((NC, L), NS, np.int32)  # NS = dummy row, dropped
    offs = np.concatenate([[0], np.cumsum(counts)])
    for c in range(NC):
        n = counts[c]
        seg = slice(offs[c], offs[c + 1])
        src_p[c, :n] = s_sorted[seg]
        dstl_p[c, :n] = d_sorted[seg] - c * NS
    return norm_src, norm_dst.reshape(NC, NS), src_p, dstl_p


def _device_impl(features, W1, b1, gamma1, beta1, W2, b2, gamma2, beta2,
                 norm_src, norm_dst_sh, src_p, dstl_p):
    import jax
    import jax.numpy as jnp

    devs = jax.devices()[:NC]
    assert len(devs) == NC

    @partial(jax.pmap, axis_name="x", devices=devs)
    def run(features, norm_src, src_p, dstl_p, norm_dst_l,
            W1, b1, g1, be1, W2, b2_, g2, be2):
        def conv(x_full, W, b, ndl):
            h = jnp.dot(x_full * norm_src[:, None], W,
                        precision=jax.lax.Precision.HIGHEST)
            msgs = h[src_p]
            agg = jnp.zeros((NS + 1, H), jnp.float32).at[dstl_p].add(msgs)[:NS]
            return jax.nn.elu(agg * ndl[:, None] + b)

        def bn(xl, gamma, beta):
            mean = jax.lax.psum(xl.sum(0), "x") / N
            var = jax.lax.psum(jnp.square(xl - mean).sum(0), "x") / N
            return (xl - mean) * jax.lax.rsqrt(var + EPS) * gamma + beta

        h1 = bn(conv(features, W1, b1, norm_dst_l), g1, be1)
        h1_full = jax.lax.all_gather(h1, "x").reshape(N, H)
        h2 = bn(conv(h1_full, W2, b2_, norm_dst_l), g2, be2)
        return h2

    rep = lambda a: np.broadcast_to(a, (NC,) + a.shape)
    out = run(rep(features), rep(norm_src), src_p, dstl_p, norm_dst_sh,
              rep(W1), rep(b1), rep(gamma1), rep(beta1),
              rep(W2), rep(b2), rep(gamma2), rep(beta2))
    return np.asarray(out).reshape(N, H)


def _host_impl(features, W1, b1, gamma1, beta1, W2, b2, gamma2, beta2,
               src, dst, norm_src, norm_dst):
    def conv(x, W, b):
        h = (x * norm_src[:, None]) @ W
        order = np.argsort(dst, kind="stable")
        d_sorted = dst[order]
        msgs = h[src[order]]
        agg = np.zeros((N, h.shape[1]), np.float32)
        starts = np.searchsorted(d_sorted, np.arange(N))
        np.add.reduceat(msgs, starts, axis=0, out=agg)
        agg[np.diff(np.concatenate([starts, [E]])) == 0] = 0
        out = agg * norm_dst[:, None] + b
        return np.where(out > 0, out, np.expm1(np.minimum(out, 0)))

    def bn(x, gamma, beta):
        mean = x.mean(0)
        var = np.square(x - mean).mean(0)
        return (x - mean) / np.sqrt(var + EPS) * gamma + beta

    h1 = bn(conv(features, W1, b1), gamma1, beta1)
    return bn(conv(h1, W2, b2), gamma2, beta2)


def kernel(features, W1, b1, gamma1, beta1, W2, b2, gamma2, beta2, src, dst):
    features = np.asarray(features, np.float32)
    W1 = np.asarray(W1, np.float32); b1 = np.asarray(b1, np.float32)
    W2 = np.asarray(W2, np.float32); b2 = np.asarray(b2, np.float32)
    gamma1 = np.asarray(gamma1, np.float32); beta1 = np.asarray(beta1, np.float32)
    gamma2 = np.asarray(gamma2, np.float32); beta2 = np.asarray(beta2, np.float32)
    src = np.asarray(src, np.int32); dst = np.asarray(dst, np.int32)

    norm_src, norm_dst_sh, src_p, dstl_p = _prep(src, dst)
    try:
        return _device_impl(features, W1, b1, gamma1, beta1, W2, b2,
                            gamma2, beta2, norm_src, norm_dst_sh, src_p, dstl_p)
    except Exception as e:  # device path unavailable -> correct host fallback
        import sys
        print(f"kernel: device path failed ({e!r}); host fallback", file=sys.stderr)
        return _host_impl(features, W1, b1, gamma1, beta1, W2, b2, gamma2,
                          beta2, src, dst, norm_src, norm_dst_sh.reshape(N))
